# revision 1
# baseline (speedup 1.0000x reference)
"""MicroExpertMoE Trainium2 kernel — bf16, wide-chunk, calibrated schedule.

vs baseline: all matmul operands bf16 (halves DMA, and measured ~8%
faster per row than fp32r on HW when rotating stationary weights), and
the expert loop uses 512-token chunks with 1024-wide down matmuls so
the per-instruction overhead (~15ns measured) is paid half as often:
  gate/up: 1024 instrs of 512 rows   (was 2048 x 256)
  down:     512 instrs of 1024 rows  (was 512 x 512 x2)
PSUM: g,u single-buffered [128,1024] (4 banks) + y 2x[128,1024]
(4 banks) = 8 exactly; the chunk pipeline (tail of chunk c emitted
after gate/up of chunk c+1) keeps PE busy while ACT/DVE run stage2.
"""

import numpy as np
import ml_dtypes

import concourse.bacc as bacc
import concourse.tile as tile
import concourse.mybir as mybir
from concourse import masks
from concourse.bass_utils import run_bass_kernel_spmd

B, T, D, E, K = 4, 2048, 1024, 16, 256
NCORES = 8
TOK = B * T // NCORES          # 1024 tokens per core
NT = TOK // 128                # 8 token tiles of 128
ND = D // 128                  # 8 d tiles
NK = K // 128                  # 2 k tiles
TC = 512                       # token chunk for expert stages
NC_CHUNK = TOK // TC           # 2 chunks
DOWN_FR = 512                  # down matmul moving width (1 PSUM bank max)
dt = mybir.dt
AF = mybir.ActivationFunctionType
ALU = mybir.AluOpType

_built = None


def _patch_hw_spec():
    """Calibrate the Tile scheduler's PE rate to the measured HW rate
    (~2.85 GHz sustained vs the model's 2.4 GHz) so the static
    instruction order it emits keeps DVE/ACT overlap valid at real
    speed ratios. Schedule-only: does not change instruction semantics."""
    import concourse.hw_specs as hw_specs
    hw_specs.TRN2Spec.PE_CYCLE = 1e9 / 2.85e9


def _emit_body(nc, pools, dram, rep):
    res, rsb, wpool, s2p, psA, psY = pools
    xt, wr, ident, y_sb = dram["xt_t"], dram["wr_t"], dram["ident"], dram["y_sb"]
    wg_d, wu_d, wd_d = dram["wg_d"], dram["wu_d"], dram["wd_d"]

    # ---------- router (batched) ----------
    lgT = rsb.tile([16, TOK], dt.float32, tag="lgT")
    for h in range(2):  # two 512-token halves
        lg_ps = psA.tile([16, 512], dt.float32, tag="gu")
        for a in range(ND):
            nc.tensor.matmul(
                lg_ps[:],
                wr[:, a * E:(a + 1) * E],
                xt[:, a * TOK + h * 512: a * TOK + (h + 1) * 512],
                start=(a == 0), stop=(a == ND - 1),
            )
        nc.vector.tensor_copy(lgT[:, h * 512:(h + 1) * 512], lg_ps[:])

    # transpose all 8 [16,128] slices into one [128, 8*16] tile
    ltr_ps = psA.tile([128, NT * E], dt.float32, tag="gu")
    for tt in range(NT):
        nc.tensor.transpose(
            ltr_ps[:, tt * E:(tt + 1) * E],
            lgT[:, tt * 128:(tt + 1) * 128], ident[:16, :16])
    lg = rsb.tile([128, NT * E], dt.float32, tag="lg")
    nc.vector.tensor_copy(lg[:], ltr_ps[:])

    def v3(ap):  # [128, 8*16] -> [128, 8, 16]
        return ap.rearrange("p (a e) -> p a e", e=E)

    def bc(ap):  # [128, 8] -> [128, 8, 16] broadcast
        return ap.unsqueeze(2).broadcast_to([128, NT, E])

    m1 = rsb.tile([128, NT], dt.float32, tag="m1")
    nc.vector.reduce_max(m1[:], v3(lg[:]), axis=mybir.AxisListType.X)
    dd = rsb.tile([128, NT * E], dt.float32, tag="dd")
    nc.vector.tensor_sub(v3(dd[:]), v3(lg[:]), bc(m1[:]))
    eq = rsb.tile([128, NT * E], dt.float32, tag="eq")
    nc.vector.tensor_scalar(eq[:], dd[:], 0.0, None, op0=ALU.is_ge)
    msk = rsb.tile([128, NT * E], dt.float32, tag="msk")
    nc.vector.scalar_tensor_tensor(msk[:], eq[:], -1e30, lg[:],
                                   op0=ALU.mult, op1=ALU.add)
    thr = rsb.tile([128, NT], dt.float32, tag="thr")
    nc.vector.reduce_max(thr[:], v3(msk[:]), axis=mybir.AxisListType.X)
    sarg = rsb.tile([128, NT * E], dt.float32, tag="sarg")
    nc.vector.tensor_sub(v3(sarg[:]), v3(lg[:]), bc(thr[:]))
    sg = rsb.tile([128, NT * E], dt.float32, tag="sg")
    nc.scalar.activation(sg[:], sarg[:], AF.Sigmoid, scale=10.0)
    l2 = rsb.tile([128, NT * E], dt.float32, tag="l2")
    nc.vector.tensor_mul(l2[:], lg[:], sg[:])
    mx = rsb.tile([128, NT], dt.float32, tag="mx")
    nc.vector.reduce_max(mx[:], v3(l2[:]), axis=mybir.AxisListType.X)
    earg = rsb.tile([128, NT * E], dt.float32, tag="earg")
    nc.vector.tensor_sub(v3(earg[:]), v3(l2[:]), bc(mx[:]))
    ex = rsb.tile([128, NT * E], dt.float32, tag="ex")
    nc.scalar.activation(ex[:], earg[:], AF.Exp)
    sm = rsb.tile([128, NT], dt.float32, tag="sm")
    nc.vector.reduce_sum(sm[:], v3(ex[:]), axis=mybir.AxisListType.X)
    rs = rsb.tile([128, NT], dt.float32, tag="rs")
    nc.vector.reciprocal(rs[:], sm[:])
    # w_all[t, tt*16+e] = softmax weight; resident for the expert loop
    w_all = rsb.tile([128, NT * E], dt.float32, tag="w_all")
    nc.vector.tensor_mul(v3(w_all[:]), v3(ex[:]), bc(rs[:]))

    # ---------- expert loop (software-pipelined) ----------
    def emit_gateup(wg_t, wu_t, c):
        g_ps = psA.tile([128, 2 * TC], dt.float32, tag="gu", name="g_ps")
        u_ps = psA.tile([128, 2 * TC], dt.float32, tag="gu", name="u_ps")
        for w_t, o_ps in ((wg_t, g_ps), (wu_t, u_ps)):
            for kt in range(NK):
                for a in range(ND):
                    nc.tensor.matmul(
                        o_ps[:, kt * TC:(kt + 1) * TC],
                        w_t[:, a * K + kt * 128: a * K + (kt + 1) * 128],
                        xt[:, a * TOK + c * TC: a * TOK + (c + 1) * TC],
                        start=(a == 0), stop=(a == ND - 1),
                    )
        return g_ps, u_ps

    def emit_tail(e, c, g_ps, u_ps, wd_t):
        # stage 2: hw = silu(g) * u   (hw cast to bf16 for the down mm)
        sg_t = s2p.tile([128, 2 * TC], dt.float32, tag="sgt", name="sg_t")
        nc.scalar.activation(sg_t[:], g_ps[:], AF.Silu)
        hw_t = s2p.tile([128, 2 * TC], dt.bfloat16, tag="hwt", name="hw_t")
        nc.vector.tensor_mul(hw_t[:], u_ps[:], sg_t[:])

        # down: y[t,d] (+)= w[t,e] * (hw[k,t].T @ WdT[k,d])
        for tt in range(TC // 128):
            gt = c * (TC // 128) + tt      # global token tile
            ys = y_sb[gt]
            w_col = w_all[:, gt * E + e: gt * E + e + 1]
            y_ps = psY.tile([128, D], dt.float32, tag="y", name="y_ps")
            for dk in range(D // DOWN_FR):
                for kt in range(NK):
                    nc.tensor.matmul(
                        y_ps[:, dk * DOWN_FR:(dk + 1) * DOWN_FR],
                        hw_t[:, kt * TC + tt * 128: kt * TC + (tt + 1) * 128],
                        wd_t[:, kt * D + dk * DOWN_FR:
                             kt * D + (dk + 1) * DOWN_FR],
                        start=(kt == 0), stop=(kt == NK - 1),
                    )
            if e == 0:
                nc.vector.tensor_scalar(ys[:], y_ps[:], w_col, None,
                                        op0=ALU.mult)
            else:
                nc.vector.scalar_tensor_tensor(ys[:], y_ps[:], w_col, ys[:],
                                               op0=ALU.mult, op1=ALU.add)

    pending = None
    for e in range(E):
        wg_t = wpool.tile([128, ND * K], dt.bfloat16, tag="wg", name="wg_t")
        nc.sync.dma_start(wg_t[:], wg_d[e])
        wu_t = wpool.tile([128, ND * K], dt.bfloat16, tag="wu", name="wu_t")
        nc.sync.dma_start(wu_t[:], wu_d[e])
        wd_t = wpool.tile([128, NK * D], dt.bfloat16, tag="wd", name="wd_t")
        nc.sync.dma_start(wd_t[:], wd_d[e])

        for c in range(NC_CHUNK):
            g_ps, u_ps = emit_gateup(wg_t, wu_t, c)
            if pending is not None:
                emit_tail(*pending)
            pending = (e, c, g_ps, u_ps, wd_t)
    emit_tail(*pending)


def _build(repeat=1):
    _patch_hw_spec()
    nc = bacc.Bacc("TRN2", target_bir_lowering=False, debug=False,
                   num_devices=NCORES)

    xt_d = nc.dram_tensor("xt", (128, ND * TOK), dt.bfloat16,
                          kind="ExternalInput").ap()
    wg_d = nc.dram_tensor("wg", (E, 128, ND * K), dt.bfloat16,
                          kind="ExternalInput").ap()
    wu_d = nc.dram_tensor("wu", (E, 128, ND * K), dt.bfloat16,
                          kind="ExternalInput").ap()
    wd_d = nc.dram_tensor("wd", (E, 128, NK * D), dt.bfloat16,
                          kind="ExternalInput").ap()
    wr_d = nc.dram_tensor("wr", (128, ND * E), dt.bfloat16,
                          kind="ExternalInput").ap()
    y_d = nc.dram_tensor("y", (TOK, D), dt.float32,
                         kind="ExternalOutput").ap()

    with tile.TileContext(nc) as tc:
        with (
            tc.tile_pool(name="resident", bufs=1) as res,
            tc.tile_pool(name="router_sb", bufs=2) as rsb,
            tc.tile_pool(name="wpool", bufs=6) as wpool,
            tc.tile_pool(name="stage2", bufs=3) as s2p,
            tc.tile_pool(name="psA", bufs=2, space="PSUM") as psA,
            tc.tile_pool(name="psY", bufs=2, space="PSUM") as psY,
        ):
            # ---------- resident loads ----------
            wr = res.tile([128, ND * E], dt.bfloat16, tag="wr")
            nc.sync.dma_start(wr[:], wr_d)
            xt = res.tile([128, ND * TOK], dt.bfloat16, tag="xt")
            for a in range(ND):
                nc.sync.dma_start(xt[:, a * TOK:(a + 1) * TOK],
                                  xt_d[:, a * TOK:(a + 1) * TOK])
            ident = res.tile([128, 128], dt.float32, tag="ident")
            masks.make_identity(nc, ident[:])
            y_sb = [res.tile([128, D], dt.float32, tag=f"ysb{i}",
                             name=f"ysb{i}") for i in range(NT)]

            pools = (res, rsb, wpool, s2p, psA, psY)
            dram = dict(xt_t=xt, wr_t=wr, ident=ident, y_sb=y_sb,
                        wg_d=wg_d, wu_d=wu_d, wd_d=wd_d)
            for _ in range(repeat):
                _emit_body(nc, pools, dram, _)

            # ---------- store ----------
            for i in range(NT):
                nc.sync.dma_start(y_d[i * 128:(i + 1) * 128, :], y_sb[i][:])

    nc.compile()
    return nc


def _prep_inputs(x, W_up, W_gate, W_down, W_router, log_temp):
    """Host-side repack: fold temp, transpose weights, partition-major,
    cast to bf16."""
    bf = ml_dtypes.bfloat16
    x = np.asarray(x, dtype=np.float32)
    W_up = np.asarray(W_up, dtype=np.float32)
    W_gate = np.asarray(W_gate, dtype=np.float32)
    W_down = np.asarray(W_down, dtype=np.float32)
    W_router = np.asarray(W_router, dtype=np.float32)
    lt = float(np.asarray(log_temp, dtype=np.float32))
    temp = float(np.log1p(np.exp(lt)) + 0.1)

    X = x.reshape(B * T, D)
    xT = np.ascontiguousarray(X.T)                       # [D, 8192]
    xt_cores = []
    for c in range(NCORES):
        sl = xT[:, c * TOK:(c + 1) * TOK]                # [1024, 1024]
        xt_cores.append(np.ascontiguousarray(
            sl.reshape(ND, 128, TOK).transpose(1, 0, 2)
            .reshape(128, ND * TOK)).astype(bf))

    def pack_dk(w):  # [E, K, D] (torch [out,in]) -> [E, 128, 8*K], d-major
        wt = w.transpose(0, 2, 1)                        # [E, D, K]
        return np.ascontiguousarray(
            wt.reshape(E, ND, 128, K).transpose(0, 2, 1, 3)
            .reshape(E, 128, ND * K)).astype(bf)

    wg_h = pack_dk(W_gate)
    wu_h = pack_dk(W_up)
    wdt = W_down.transpose(0, 2, 1)                      # [E, K, D]
    wd_h = np.ascontiguousarray(
        wdt.reshape(E, NK, 128, D).transpose(0, 2, 1, 3)
        .reshape(E, 128, NK * D)).astype(bf)
    wrt = np.ascontiguousarray(W_router.T) / temp        # [D, E]
    wr_h = np.ascontiguousarray(
        wrt.reshape(ND, 128, E).transpose(1, 0, 2)
        .reshape(128, ND * E)).astype(bf)

    in_maps = []
    for c in range(NCORES):
        in_maps.append({
            "xt": xt_cores[c],
            "wg": wg_h, "wu": wu_h, "wd": wd_h, "wr": wr_h,
        })
    return in_maps


def kernel(x, W_up, W_gate, W_down, W_router, log_temp, _trace=False):
    global _built
    if _built is None:
        _built = _build()
    nc = _built
    in_maps = _prep_inputs(x, W_up, W_gate, W_down, W_router, log_temp)
    res = run_bass_kernel_spmd(nc, in_maps, core_ids=list(range(NCORES)),
                               trace=_trace)
    out = np.empty((B * T, D), dtype=np.float32)
    for c in range(NCORES):
        out[c * TOK:(c + 1) * TOK, :] = res.results[c]["y"]
    kernel.last_results = res
    return out.reshape(B, T, D)



# revision 3
# speedup vs baseline: 1.1840x; 1.1840x over previous
"""MicroExpertMoE Trainium2 kernel — bf16, data-parallel over tokens.

Each of the 8 cores handles 1024 tokens x all 16 experts; weights stream
from HBM once (24MB) while x (2MB) stays resident.  The PE stream
(16 experts x 2 chunks x [32 gate/up + 16 down] N=512 bf16 matmuls =
1536 MMs) is the measured roofline; all DVE/ACT/DMA work hides under it
(a pure-MM skeleton of the same stream measures within ~7us, which is
the router's PE share).

Structure notes:
 - expert-0 chunk-0 gate/up is emitted BEFORE the router, so the PE
   starts as soon as x tile 0 + Wg[0] land (~1.5us) instead of after the
   full x DMA (5.6us); the router (which needs all of x) then runs while
   later loads are covered.
 - router PSUM borrows the psY rotation (those tiles are dead before the
   first down-tail needs them).
 - chunk pipeline: the tail (silu*up, down matmuls, weighted apply) of
   chunk c is emitted after gate/up of chunk c+1, giving the stage-2
   ACT/DVE ops a ~3.4us runway so single-buffered g/u PSUM never stalls
   the PE.
 - down PSUM is one bank per 512-wide half with the DVE apply issued per
   half, so the apply drain never blocks PE reuse of the bank.
 - y is produced in bf16: expert 15's apply writes a bf16 tile directly
   (DVE output cast) and each token tile is DMA'd as soon as its last
   apply finishes -- an ~11us exposed fp32 store tail becomes ~2us.
   The host upcasts to fp32 (adds <=2^-9 rounding, well inside the
   2e-2 budget; measured rel err 6.1e-3 vs 5.6e-3 for fp32 out).
"""

import numpy as np
import ml_dtypes

import concourse.bacc as bacc
import concourse.tile as tile
import concourse.mybir as mybir
from concourse import masks
from concourse.bass_utils import run_bass_kernel_spmd

B, T, D, E, K = 4, 2048, 1024, 16, 256
NCORES = 8
TOK = B * T // NCORES          # 1024 tokens per core
NT = TOK // 128                # 8 token tiles of 128
ND = D // 128                  # 8 d tiles
NK = K // 128                  # 2 k tiles
TC = 512                       # token chunk for expert stages
NC_CHUNK = TOK // TC           # 2 chunks
DOWN_FR = 512                  # down matmul moving width (1 PSUM bank max)
dt = mybir.dt
AF = mybir.ActivationFunctionType
ALU = mybir.AluOpType

_built = None


def _patch_hw_spec():
    pass


def _emit_body(nc, pools, dram, rep):
    res, rsb, wpool, s2p, psA, psY = pools
    xt, wr, ident, y_sb = dram["xt_t"], dram["wr_t"], dram["ident"], dram["y_sb"]
    wg_d, wu_d, wd_d = dram["wg_d"], dram["wu_d"], dram["wd_d"]
    y_d = dram["y_d"]

    # ---------- expert-stage emitters ----------
    def emit_gateup(wg_t, wu_t, c):
        g_ps = psA.tile([128, 2 * TC], dt.float32, tag="gu", name="g_ps")
        u_ps = psA.tile([128, 2 * TC], dt.float32, tag="gu", name="u_ps")
        for w_t, o_ps in ((wg_t, g_ps), (wu_t, u_ps)):
            for kt in range(NK):
                for a in range(ND):
                    nc.tensor.matmul(
                        o_ps[:, kt * TC:(kt + 1) * TC],
                        w_t[:, a * K + kt * 128: a * K + (kt + 1) * 128],
                        xt[:, a * TOK + c * TC: a * TOK + (c + 1) * TC],
                        start=(a == 0), stop=(a == ND - 1),
                    )
        return g_ps, u_ps

    def emit_tail(e, c, g_ps, u_ps, wd_t, w_all):
        # stage 2: hw = silu(g) * u   (hw cast to bf16 for the down mm)
        sg_t = s2p.tile([128, 2 * TC], dt.float32, tag="sgt", name="sg_t")
        nc.scalar.activation(sg_t[:], g_ps[:], AF.Silu)
        hw_t = s2p.tile([128, 2 * TC], dt.bfloat16, tag="hwt", name="hw_t")
        nc.vector.tensor_mul(hw_t[:], u_ps[:], sg_t[:])

        # down: y[t,d] (+)= w[t,e] * (hw[k,t].T @ WdT[k,d])
        # one PSUM bank per dk-half, applied as soon as its 2 matmuls stop,
        # so the DVE drain never blocks the PE on psY reuse
        for tt in range(TC // 128):
            gt = c * (TC // 128) + tt      # global token tile
            ys = y_sb[gt]
            w_col = w_all[:, gt * E + e: gt * E + e + 1]
            y_bf = None
            if e == E - 1:
                y_bf = s2p.tile([128, D], dt.bfloat16, tag="ybf", name="y_bf")
            for dk in range(D // DOWN_FR):
                y_ps = psY.tile([128, DOWN_FR], dt.float32, tag="y",
                                name="y_ps")
                for kt in range(NK):
                    nc.tensor.matmul(
                        y_ps[:],
                        hw_t[:, kt * TC + tt * 128: kt * TC + (tt + 1) * 128],
                        wd_t[:, kt * D + dk * DOWN_FR:
                             kt * D + (dk + 1) * DOWN_FR],
                        start=(kt == 0), stop=(kt == NK - 1),
                    )
                sl = slice(dk * DOWN_FR, (dk + 1) * DOWN_FR)
                if e == 0:
                    nc.vector.tensor_scalar(ys[:, sl], y_ps[:], w_col, None,
                                            op0=ALU.mult)
                elif e == E - 1:
                    nc.vector.scalar_tensor_tensor(y_bf[:, sl], y_ps[:],
                                                   w_col, ys[:, sl],
                                                   op0=ALU.mult, op1=ALU.add)
                else:
                    nc.vector.scalar_tensor_tensor(ys[:, sl], y_ps[:], w_col,
                                                   ys[:, sl],
                                                   op0=ALU.mult, op1=ALU.add)
            if e == E - 1:
                nc.sync.dma_start(y_d[gt * 128:(gt + 1) * 128, :], y_bf[:])

    # ---------- expert 0 chunk 0 gate/up first (needs only x tiles + Wg/Wu
    # of expert 0, so PE starts ~1.5us in while the rest of x streams) ----
    wts = {}
    for e in (0, 1):
        wg_t = wpool.tile([128, ND * K], dt.bfloat16, tag="wg", name="wg_t")
        nc.sync.dma_start(wg_t[:], wg_d[e])
        wu_t = wpool.tile([128, ND * K], dt.bfloat16, tag="wu", name="wu_t")
        nc.sync.dma_start(wu_t[:], wu_d[e])
        wd_t = wpool.tile([128, NK * D], dt.bfloat16, tag="wd", name="wd_t")
        nc.sync.dma_start(wd_t[:], wd_d[e])
        wts[e] = (wg_t, wu_t, wd_t)

    pending = (0, 0) + emit_gateup(wts[0][0], wts[0][1], 0) + (wts[0][2],)

    # ---------- router (batched), PSUM borrowed from psY's rotation ----
    lgT = rsb.tile([16, TOK], dt.float32, tag="lgT")
    for h in range(2):  # two 512-token halves
        lg_ps = psY.tile([128, DOWN_FR], dt.float32, tag="y", name="lg_ps")
        for a in range(ND):
            nc.tensor.matmul(
                lg_ps[:16, :512],
                wr[:, a * E:(a + 1) * E],
                xt[:, a * TOK + h * 512: a * TOK + (h + 1) * 512],
                start=(a == 0), stop=(a == ND - 1),
            )
        nc.vector.tensor_copy(lgT[:, h * 512:(h + 1) * 512], lg_ps[:16, :])

    # transpose all 8 [16,128] slices into one [128, 8*16] tile
    ltr_ps = psY.tile([128, DOWN_FR], dt.float32, tag="y", name="ltr_ps")
    for tt in range(NT):
        nc.tensor.transpose(
            ltr_ps[:, tt * E:(tt + 1) * E],
            lgT[:, tt * 128:(tt + 1) * 128], ident[:16, :16])
    lg = rsb.tile([128, NT * E], dt.float32, tag="lg")
    nc.vector.tensor_copy(lg[:], ltr_ps[:, :NT * E])

    def v3(ap):  # [128, 8*16] -> [128, 8, 16]
        return ap.rearrange("p (a e) -> p a e", e=E)

    def bc(ap):  # [128, 8] -> [128, 8, 16] broadcast
        return ap.unsqueeze(2).broadcast_to([128, NT, E])

    m1 = rsb.tile([128, NT], dt.float32, tag="m1")
    nc.vector.reduce_max(m1[:], v3(lg[:]), axis=mybir.AxisListType.X)
    dd = rsb.tile([128, NT * E], dt.float32, tag="dd")
    nc.vector.tensor_sub(v3(dd[:]), v3(lg[:]), bc(m1[:]))
    eq = rsb.tile([128, NT * E], dt.float32, tag="eq")
    nc.vector.tensor_scalar(eq[:], dd[:], 0.0, None, op0=ALU.is_ge)
    msk = rsb.tile([128, NT * E], dt.float32, tag="msk")
    nc.vector.scalar_tensor_tensor(msk[:], eq[:], -1e30, lg[:],
                                   op0=ALU.mult, op1=ALU.add)
    thr = rsb.tile([128, NT], dt.float32, tag="thr")
    nc.vector.reduce_max(thr[:], v3(msk[:]), axis=mybir.AxisListType.X)
    sarg = rsb.tile([128, NT * E], dt.float32, tag="sarg")
    nc.vector.tensor_sub(v3(sarg[:]), v3(lg[:]), bc(thr[:]))
    sg = rsb.tile([128, NT * E], dt.float32, tag="sg")
    nc.scalar.activation(sg[:], sarg[:], AF.Sigmoid, scale=10.0)
    l2 = rsb.tile([128, NT * E], dt.float32, tag="l2")
    nc.vector.tensor_mul(l2[:], lg[:], sg[:])
    mx = rsb.tile([128, NT], dt.float32, tag="mx")
    nc.vector.reduce_max(mx[:], v3(l2[:]), axis=mybir.AxisListType.X)
    earg = rsb.tile([128, NT * E], dt.float32, tag="earg")
    nc.vector.tensor_sub(v3(earg[:]), v3(l2[:]), bc(mx[:]))
    ex = rsb.tile([128, NT * E], dt.float32, tag="ex")
    nc.scalar.activation(ex[:], earg[:], AF.Exp)
    sm = rsb.tile([128, NT], dt.float32, tag="sm")
    nc.vector.reduce_sum(sm[:], v3(ex[:]), axis=mybir.AxisListType.X)
    rs = rsb.tile([128, NT], dt.float32, tag="rs")
    nc.vector.reciprocal(rs[:], sm[:])
    # w_all[t, tt*16+e] = softmax weight; resident for the expert loop
    w_all = rsb.tile([128, NT * E], dt.float32, tag="w_all")
    nc.vector.tensor_mul(v3(w_all[:]), v3(ex[:]), bc(rs[:]))

    # ---------- expert loop (software-pipelined) ----------
    for e in range(E):
        if e >= 2:
            wg_t = wpool.tile([128, ND * K], dt.bfloat16, tag="wg",
                              name="wg_t")
            nc.sync.dma_start(wg_t[:], wg_d[e])
            wu_t = wpool.tile([128, ND * K], dt.bfloat16, tag="wu",
                              name="wu_t")
            nc.sync.dma_start(wu_t[:], wu_d[e])
            wd_t = wpool.tile([128, NK * D], dt.bfloat16, tag="wd",
                              name="wd_t")
            nc.sync.dma_start(wd_t[:], wd_d[e])
        else:
            wg_t, wu_t, wd_t = wts[e]

        for c in range(NC_CHUNK):
            if e == 0 and c == 0:
                continue  # emitted above
            g_ps, u_ps = emit_gateup(wg_t, wu_t, c)
            if pending is not None:
                emit_tail(*pending, w_all)
            pending = (e, c, g_ps, u_ps, wd_t)
    emit_tail(*pending, w_all)


def _build(repeat=1, loop=1):
    _patch_hw_spec()
    nc = bacc.Bacc("TRN2", target_bir_lowering=False, debug=False,
                   num_devices=NCORES)

    xt_d = nc.dram_tensor("xt", (128, ND * TOK), dt.bfloat16,
                          kind="ExternalInput").ap()
    wg_d = nc.dram_tensor("wg", (E, 128, ND * K), dt.bfloat16,
                          kind="ExternalInput").ap()
    wu_d = nc.dram_tensor("wu", (E, 128, ND * K), dt.bfloat16,
                          kind="ExternalInput").ap()
    wd_d = nc.dram_tensor("wd", (E, 128, NK * D), dt.bfloat16,
                          kind="ExternalInput").ap()
    wr_d = nc.dram_tensor("wr", (128, ND * E), dt.bfloat16,
                          kind="ExternalInput").ap()
    y_d = nc.dram_tensor("y", (TOK, D), dt.bfloat16,
                         kind="ExternalOutput").ap()

    with tile.TileContext(nc) as tc:
        with (
            tc.tile_pool(name="resident", bufs=1) as res,
            tc.tile_pool(name="router_sb", bufs=2) as rsb,
            tc.tile_pool(name="wpool", bufs=6) as wpool,
            tc.tile_pool(name="stage2", bufs=3) as s2p,
            tc.tile_pool(name="psA", bufs=2, space="PSUM") as psA,
            tc.tile_pool(name="psY", bufs=4, space="PSUM") as psY,
        ):
            # ---------- resident loads ----------
            wr = res.tile([128, ND * E], dt.bfloat16, tag="wr")
            nc.sync.dma_start(wr[:], wr_d)
            xt = res.tile([128, ND * TOK], dt.bfloat16, tag="xt")
            for a in range(ND):
                nc.sync.dma_start(xt[:, a * TOK:(a + 1) * TOK],
                                  xt_d[:, a * TOK:(a + 1) * TOK])
            ident = res.tile([128, 128], dt.float32, tag="ident")
            masks.make_identity(nc, ident[:])
            y_sb = [res.tile([128, D], dt.float32, tag=f"ysb{i}",
                             name=f"ysb{i}") for i in range(NT)]

            pools = (res, rsb, wpool, s2p, psA, psY)
            dram = dict(xt_t=xt, wr_t=wr, ident=ident, y_sb=y_sb,
                        wg_d=wg_d, wu_d=wu_d, wd_d=wd_d, y_d=y_d)
            if loop > 1:
                with tc.For_i(0, loop, 1):
                    for _ in range(repeat):
                        _emit_body(nc, pools, dram, _)
            else:
                for _ in range(repeat):
                    _emit_body(nc, pools, dram, _)

    nc.compile()
    return nc


def _prep_inputs(x, W_up, W_gate, W_down, W_router, log_temp):
    """Host-side repack: fold temp, transpose weights, partition-major,
    cast to bf16."""
    bf = ml_dtypes.bfloat16
    x = np.asarray(x, dtype=np.float32)
    W_up = np.asarray(W_up, dtype=np.float32)
    W_gate = np.asarray(W_gate, dtype=np.float32)
    W_down = np.asarray(W_down, dtype=np.float32)
    W_router = np.asarray(W_router, dtype=np.float32)
    lt = float(np.asarray(log_temp, dtype=np.float32))
    temp = float(np.log1p(np.exp(lt)) + 0.1)

    X = x.reshape(B * T, D)
    xT = np.ascontiguousarray(X.T)                       # [D, 8192]
    xt_cores = []
    for c in range(NCORES):
        sl = xT[:, c * TOK:(c + 1) * TOK]                # [1024, 1024]
        xt_cores.append(np.ascontiguousarray(
            sl.reshape(ND, 128, TOK).transpose(1, 0, 2)
            .reshape(128, ND * TOK)).astype(bf))

    def pack_dk(w):  # [E, K, D] (torch [out,in]) -> [E, 128, 8*K], d-major
        wt = w.transpose(0, 2, 1)                        # [E, D, K]
        return np.ascontiguousarray(
            wt.reshape(E, ND, 128, K).transpose(0, 2, 1, 3)
            .reshape(E, 128, ND * K)).astype(bf)

    wg_h = pack_dk(W_gate)
    wu_h = pack_dk(W_up)
    wdt = W_down.transpose(0, 2, 1)                      # [E, K, D]
    wd_h = np.ascontiguousarray(
        wdt.reshape(E, NK, 128, D).transpose(0, 2, 1, 3)
        .reshape(E, 128, NK * D)).astype(bf)
    wrt = np.ascontiguousarray(W_router.T) / temp        # [D, E]
    wr_h = np.ascontiguousarray(
        wrt.reshape(ND, 128, E).transpose(1, 0, 2)
        .reshape(128, ND * E)).astype(bf)

    in_maps = []
    for c in range(NCORES):
        in_maps.append({
            "xt": xt_cores[c],
            "wg": wg_h, "wu": wu_h, "wd": wd_h, "wr": wr_h,
        })
    return in_maps


def kernel(x, W_up, W_gate, W_down, W_router, log_temp, _trace=False):
    global _built
    if _built is None:
        _built = _build()
    nc = _built
    in_maps = _prep_inputs(x, W_up, W_gate, W_down, W_router, log_temp)
    res = run_bass_kernel_spmd(nc, in_maps, core_ids=list(range(NCORES)),
                               trace=_trace)
    out = np.empty((B * T, D), dtype=np.float32)
    for c in range(NCORES):
        out[c * TOK:(c + 1) * TOK, :] = res.results[c]["y"].astype(np.float32)
    kernel.last_results = res
    return out.reshape(B, T, D)


# revision 6
# speedup vs baseline: 1.1877x; 1.0031x over previous
"""MicroExpertMoE Trainium2 kernel — bf16, data-parallel over tokens.

Each of the 8 cores handles 1024 tokens x all 16 experts; weights stream
from HBM once (24MB) while x (2MB) stays resident.  The PE stream
(16 experts x 2 chunks x [32 gate/up + 16 down] N=512 bf16 matmuls =
1536 MMs) is the measured roofline; all DVE/ACT/DMA work hides under it
(a pure-MM skeleton of the same stream measures within ~7us, which is
the router's PE share).

Structure notes:
 - expert-0 chunk-0 gate/up is emitted BEFORE the router, so the PE
   starts as soon as x tile 0 + Wg[0] land (~1.5us) instead of after the
   full x DMA (5.6us); the router (which needs all of x) then runs while
   later loads are covered.
 - router PSUM borrows the psY rotation (those tiles are dead before the
   first down-tail needs them).
 - chunk pipeline: the tail (silu*up, down matmuls, weighted apply) of
   chunk c is emitted after gate/up of chunk c+1, giving the stage-2
   ACT/DVE ops a ~3.4us runway so single-buffered g/u PSUM never stalls
   the PE.
 - down PSUM is one bank per 512-wide half with the DVE apply issued per
   half, so the apply drain never blocks PE reuse of the bank.
 - y is produced in bf16: expert 15's apply writes a bf16 tile directly
   (DVE output cast) and each token tile is DMA'd as soon as its last
   apply finishes -- an ~11us exposed fp32 store tail becomes ~2us.
   The host upcasts to fp32 (adds <=2^-9 rounding, well inside the
   2e-2 budget; measured rel err 6.1e-3 vs 5.6e-3 for fp32 out).
"""

import numpy as np
import ml_dtypes

import concourse.bacc as bacc
import concourse.tile as tile
import concourse.mybir as mybir
from concourse import masks
from concourse.bass_utils import run_bass_kernel_spmd

B, T, D, E, K = 4, 2048, 1024, 16, 256
NCORES = 8
TOK = B * T // NCORES          # 1024 tokens per core
NT = TOK // 128                # 8 token tiles of 128
ND = D // 128                  # 8 d tiles
NK = K // 128                  # 2 k tiles
TC = 512                       # token chunk for expert stages
NC_CHUNK = TOK // TC           # 2 chunks
DOWN_FR = 512                  # down matmul moving width (1 PSUM bank max)
dt = mybir.dt
AF = mybir.ActivationFunctionType
ALU = mybir.AluOpType

_built = None


def _patch_hw_spec():
    pass


def _emit_body(nc, pools, dram, rep):
    res, rsb, wpool, s2p, psA, psY = pools
    xt, wr, ident, y_sb = dram["xt_t"], dram["wr_t"], dram["ident"], dram["y_sb"]
    wg_d, wu_d, wd_d = dram["wg_d"], dram["wu_d"], dram["wd_d"]
    y_d = dram["y_d"]

    # ---------- expert-stage emitters ----------
    def emit_gateup(wg_t, wu_t, c):
        g_ps = psA.tile([128, 2 * TC], dt.float32, tag="gu", name="g_ps")
        u_ps = psA.tile([128, 2 * TC], dt.float32, tag="gu", name="u_ps")
        for w_t, o_ps in ((wg_t, g_ps), (wu_t, u_ps)):
            for kt in range(NK):
                for a in range(ND):
                    nc.tensor.matmul(
                        o_ps[:, kt * TC:(kt + 1) * TC],
                        w_t[:, a * K + kt * 128: a * K + (kt + 1) * 128],
                        xt[:, a * TOK + c * TC: a * TOK + (c + 1) * TC],
                        start=(a == 0), stop=(a == ND - 1),
                    )
        return g_ps, u_ps

    def emit_tail(e, c, g_ps, u_ps, wd_t, w_all):
        # stage 2: hw = silu(g) * u   (hw cast to bf16 for the down mm)
        sg_t = s2p.tile([128, 2 * TC], dt.float32, tag="sgt", name="sg_t")
        nc.scalar.activation(sg_t[:], g_ps[:], AF.Silu)
        hw_t = s2p.tile([128, 2 * TC], dt.bfloat16, tag="hwt", name="hw_t")
        nc.vector.tensor_mul(hw_t[:], u_ps[:], sg_t[:])

        # down: y[t,d] (+)= w[t,e] * (hw[k,t].T @ WdT[k,d])
        # one PSUM bank per dk-half, applied as soon as its 2 matmuls stop,
        # so the DVE drain never blocks the PE on psY reuse
        for tt in range(TC // 128):
            gt = c * (TC // 128) + tt      # global token tile
            ys = y_sb[gt]
            w_col = w_all[:, gt * E + e: gt * E + e + 1]
            y_bf = None
            if e == E - 1:
                y_bf = s2p.tile([128, D], dt.bfloat16, tag="ybf", name="y_bf")
            for dk in range(D // DOWN_FR):
                y_ps = psY.tile([128, DOWN_FR], dt.float32, tag="y",
                                name="y_ps")
                for kt in range(NK):
                    nc.tensor.matmul(
                        y_ps[:],
                        hw_t[:, kt * TC + tt * 128: kt * TC + (tt + 1) * 128],
                        wd_t[:, kt * D + dk * DOWN_FR:
                             kt * D + (dk + 1) * DOWN_FR],
                        start=(kt == 0), stop=(kt == NK - 1),
                    )
                sl = slice(dk * DOWN_FR, (dk + 1) * DOWN_FR)
                if e == 0:
                    nc.vector.tensor_scalar(ys[:, sl], y_ps[:], w_col, None,
                                            op0=ALU.mult)
                elif e == E - 1:
                    nc.vector.scalar_tensor_tensor(y_bf[:, sl], y_ps[:],
                                                   w_col, ys[:, sl],
                                                   op0=ALU.mult, op1=ALU.add)
                else:
                    nc.vector.scalar_tensor_tensor(ys[:, sl], y_ps[:], w_col,
                                                   ys[:, sl],
                                                   op0=ALU.mult, op1=ALU.add)
            if e == E - 1:
                nc.sync.dma_start(y_d[gt * 128:(gt + 1) * 128, :], y_bf[:])

    # ---------- expert 0 chunk 0 gate/up first (needs only x tiles + Wg/Wu
    # of expert 0, so PE starts ~1.5us in while the rest of x streams) ----
    wts = {}
    for e in (0, 1):
        wg_t = wpool.tile([128, ND * K], dt.bfloat16, tag="wg", name="wg_t")
        nc.sync.dma_start(wg_t[:], wg_d[e])
        wu_t = wpool.tile([128, ND * K], dt.bfloat16, tag="wu", name="wu_t")
        nc.sync.dma_start(wu_t[:], wu_d[e])
        wd_t = wpool.tile([128, NK * D], dt.bfloat16, tag="wd", name="wd_t")
        nc.sync.dma_start(wd_t[:], wd_d[e])
        wts[e] = (wg_t, wu_t, wd_t)

    pending = (0, 0) + emit_gateup(wts[0][0], wts[0][1], 0) + (wts[0][2],)

    # ---------- router (batched), PSUM borrowed from psY's rotation ----
    # two col-group-tiled partial sums (4 accumulation steps each) run
    # CONCURRENTLY in different 32-col groups of the PE array, halving the
    # streamed column count; one DVE add combines them.
    lgT = rsb.tile([16, TOK], dt.float32, tag="lgT")
    for h in range(2):  # two 512-token halves
        lg_a = psY.tile([128, DOWN_FR], dt.float32, tag="y", name="lg_a")
        lg_b = psY.tile([128, DOWN_FR], dt.float32, tag="y", name="lg_b")
        for step in range(ND // 2):
            for g, (ps, base) in enumerate(((lg_a, 0), (lg_b, 32))):
                a = step + g * (ND // 2)
                nc.tensor.matmul(
                    ps[base:base + 16, :512],
                    wr[:, a * E:(a + 1) * E],
                    xt[:, a * TOK + h * 512: a * TOK + (h + 1) * 512],
                    start=(step == 0), stop=(step == ND // 2 - 1),
                    tile_position=(0, base),
                )
        lgB = rsb.tile([16, 512], dt.float32, tag="lgB", name="lgB")
        nc.scalar.copy(lgB[:], lg_b[32:48, :])
        nc.vector.tensor_add(lgT[:, h * 512:(h + 1) * 512],
                             lg_a[0:16, :], lgB[:])

    # transpose all 8 [16,128] slices into one [128, 8*16] tile
    ltr_ps = psY.tile([128, DOWN_FR], dt.float32, tag="y", name="ltr_ps")
    for tt in range(NT):
        nc.tensor.transpose(
            ltr_ps[:, tt * E:(tt + 1) * E],
            lgT[:, tt * 128:(tt + 1) * 128], ident[:16, :16])
    lg = rsb.tile([128, NT * E], dt.float32, tag="lg")
    nc.vector.tensor_copy(lg[:], ltr_ps[:, :NT * E])

    def v3(ap):  # [128, 8*16] -> [128, 8, 16]
        return ap.rearrange("p (a e) -> p a e", e=E)

    def bc(ap):  # [128, 8] -> [128, 8, 16] broadcast
        return ap.unsqueeze(2).broadcast_to([128, NT, E])

    m1 = rsb.tile([128, NT], dt.float32, tag="m1")
    nc.vector.reduce_max(m1[:], v3(lg[:]), axis=mybir.AxisListType.X)
    dd = rsb.tile([128, NT * E], dt.float32, tag="dd")
    nc.vector.tensor_sub(v3(dd[:]), v3(lg[:]), bc(m1[:]))
    eq = rsb.tile([128, NT * E], dt.float32, tag="eq")
    nc.vector.tensor_scalar(eq[:], dd[:], 0.0, None, op0=ALU.is_ge)
    msk = rsb.tile([128, NT * E], dt.float32, tag="msk")
    nc.vector.scalar_tensor_tensor(msk[:], eq[:], -1e30, lg[:],
                                   op0=ALU.mult, op1=ALU.add)
    thr = rsb.tile([128, NT], dt.float32, tag="thr")
    nc.vector.reduce_max(thr[:], v3(msk[:]), axis=mybir.AxisListType.X)
    sarg = rsb.tile([128, NT * E], dt.float32, tag="sarg")
    nc.vector.tensor_sub(v3(sarg[:]), v3(lg[:]), bc(thr[:]))
    sg = rsb.tile([128, NT * E], dt.float32, tag="sg")
    nc.scalar.activation(sg[:], sarg[:], AF.Sigmoid, scale=10.0)
    l2 = rsb.tile([128, NT * E], dt.float32, tag="l2")
    nc.vector.tensor_mul(l2[:], lg[:], sg[:])
    mx = rsb.tile([128, NT], dt.float32, tag="mx")
    nc.vector.reduce_max(mx[:], v3(l2[:]), axis=mybir.AxisListType.X)
    earg = rsb.tile([128, NT * E], dt.float32, tag="earg")
    nc.vector.tensor_sub(v3(earg[:]), v3(l2[:]), bc(mx[:]))
    ex = rsb.tile([128, NT * E], dt.float32, tag="ex")
    nc.scalar.activation(ex[:], earg[:], AF.Exp)
    sm = rsb.tile([128, NT], dt.float32, tag="sm")
    nc.vector.reduce_sum(sm[:], v3(ex[:]), axis=mybir.AxisListType.X)
    rs = rsb.tile([128, NT], dt.float32, tag="rs")
    nc.vector.reciprocal(rs[:], sm[:])
    # w_all[t, tt*16+e] = softmax weight; resident for the expert loop
    w_all = rsb.tile([128, NT * E], dt.float32, tag="w_all")
    nc.vector.tensor_mul(v3(w_all[:]), v3(ex[:]), bc(rs[:]))

    # ---------- expert loop (software-pipelined) ----------
    for e in range(E):
        if e >= 2:
            wg_t = wpool.tile([128, ND * K], dt.bfloat16, tag="wg",
                              name="wg_t")
            nc.sync.dma_start(wg_t[:], wg_d[e])
            wu_t = wpool.tile([128, ND * K], dt.bfloat16, tag="wu",
                              name="wu_t")
            nc.sync.dma_start(wu_t[:], wu_d[e])
            wd_t = wpool.tile([128, NK * D], dt.bfloat16, tag="wd",
                              name="wd_t")
            nc.sync.dma_start(wd_t[:], wd_d[e])
        else:
            wg_t, wu_t, wd_t = wts[e]

        for c in range(NC_CHUNK):
            if e == 0 and c == 0:
                continue  # emitted above
            g_ps, u_ps = emit_gateup(wg_t, wu_t, c)
            if pending is not None:
                emit_tail(*pending, w_all)
            pending = (e, c, g_ps, u_ps, wd_t)
    emit_tail(*pending, w_all)


def _build(repeat=1, loop=1):
    _patch_hw_spec()
    nc = bacc.Bacc("TRN2", target_bir_lowering=False, debug=False,
                   num_devices=NCORES)

    xt_d = nc.dram_tensor("xt", (128, ND * TOK), dt.bfloat16,
                          kind="ExternalInput").ap()
    wg_d = nc.dram_tensor("wg", (E, 128, ND * K), dt.bfloat16,
                          kind="ExternalInput").ap()
    wu_d = nc.dram_tensor("wu", (E, 128, ND * K), dt.bfloat16,
                          kind="ExternalInput").ap()
    wd_d = nc.dram_tensor("wd", (E, 128, NK * D), dt.bfloat16,
                          kind="ExternalInput").ap()
    wr_d = nc.dram_tensor("wr", (128, ND * E), dt.bfloat16,
                          kind="ExternalInput").ap()
    y_d = nc.dram_tensor("y", (TOK, D), dt.bfloat16,
                         kind="ExternalOutput").ap()

    with tile.TileContext(nc) as tc:
        with (
            tc.tile_pool(name="resident", bufs=1) as res,
            tc.tile_pool(name="router_sb", bufs=2) as rsb,
            tc.tile_pool(name="wpool", bufs=6) as wpool,
            tc.tile_pool(name="stage2", bufs=3) as s2p,
            tc.tile_pool(name="psA", bufs=2, space="PSUM") as psA,
            tc.tile_pool(name="psY", bufs=4, space="PSUM") as psY,
        ):
            # ---------- resident loads ----------
            wr = res.tile([128, ND * E], dt.bfloat16, tag="wr")
            nc.sync.dma_start(wr[:], wr_d)
            xt = res.tile([128, ND * TOK], dt.bfloat16, tag="xt")
            for a in range(ND):
                nc.sync.dma_start(xt[:, a * TOK:(a + 1) * TOK],
                                  xt_d[:, a * TOK:(a + 1) * TOK])
            ident = res.tile([128, 128], dt.float32, tag="ident")
            masks.make_identity(nc, ident[:])
            y_sb = [res.tile([128, D], dt.float32, tag=f"ysb{i}",
                             name=f"ysb{i}") for i in range(NT)]

            pools = (res, rsb, wpool, s2p, psA, psY)
            dram = dict(xt_t=xt, wr_t=wr, ident=ident, y_sb=y_sb,
                        wg_d=wg_d, wu_d=wu_d, wd_d=wd_d, y_d=y_d)
            if loop > 1:
                with tc.For_i(0, loop, 1):
                    for _ in range(repeat):
                        _emit_body(nc, pools, dram, _)
            else:
                for _ in range(repeat):
                    _emit_body(nc, pools, dram, _)

    nc.compile()
    return nc


def _prep_inputs(x, W_up, W_gate, W_down, W_router, log_temp):
    """Host-side repack: fold temp, transpose weights, partition-major,
    cast to bf16."""
    bf = ml_dtypes.bfloat16
    x = np.asarray(x, dtype=np.float32)
    W_up = np.asarray(W_up, dtype=np.float32)
    W_gate = np.asarray(W_gate, dtype=np.float32)
    W_down = np.asarray(W_down, dtype=np.float32)
    W_router = np.asarray(W_router, dtype=np.float32)
    lt = float(np.asarray(log_temp, dtype=np.float32))
    temp = float(np.log1p(np.exp(lt)) + 0.1)

    X = x.reshape(B * T, D)
    xT = np.ascontiguousarray(X.T)                       # [D, 8192]
    xt_cores = []
    for c in range(NCORES):
        sl = xT[:, c * TOK:(c + 1) * TOK]                # [1024, 1024]
        xt_cores.append(np.ascontiguousarray(
            sl.reshape(ND, 128, TOK).transpose(1, 0, 2)
            .reshape(128, ND * TOK)).astype(bf))

    def pack_dk(w):  # [E, K, D] (torch [out,in]) -> [E, 128, 8*K], d-major
        wt = w.transpose(0, 2, 1)                        # [E, D, K]
        return np.ascontiguousarray(
            wt.reshape(E, ND, 128, K).transpose(0, 2, 1, 3)
            .reshape(E, 128, ND * K)).astype(bf)

    wg_h = pack_dk(W_gate)
    wu_h = pack_dk(W_up)
    wdt = W_down.transpose(0, 2, 1)                      # [E, K, D]
    wd_h = np.ascontiguousarray(
        wdt.reshape(E, NK, 128, D).transpose(0, 2, 1, 3)
        .reshape(E, 128, NK * D)).astype(bf)
    wrt = np.ascontiguousarray(W_router.T) / temp        # [D, E]
    wr_h = np.ascontiguousarray(
        wrt.reshape(ND, 128, E).transpose(1, 0, 2)
        .reshape(128, ND * E)).astype(bf)

    in_maps = []
    for c in range(NCORES):
        in_maps.append({
            "xt": xt_cores[c],
            "wg": wg_h, "wu": wu_h, "wd": wd_h, "wr": wr_h,
        })
    return in_maps


def kernel(x, W_up, W_gate, W_down, W_router, log_temp, _trace=False):
    global _built
    if _built is None:
        _built = _build()
    nc = _built
    in_maps = _prep_inputs(x, W_up, W_gate, W_down, W_router, log_temp)
    res = run_bass_kernel_spmd(nc, in_maps, core_ids=list(range(NCORES)),
                               trace=_trace)
    out = np.empty((B * T, D), dtype=np.float32)
    for c in range(NCORES):
        out[c * TOK:(c + 1) * TOK, :] = res.results[c]["y"].astype(np.float32)
    kernel.last_results = res
    return out.reshape(B, T, D)


# revision 7
# speedup vs baseline: 1.1907x; 1.0025x over previous
"""MicroExpertMoE Trainium2 kernel — bf16, data-parallel over tokens.

Each of the 8 cores handles 1024 tokens x all 16 experts; weights stream
from HBM once (24MB) while x (2MB) stays resident.  The PE stream
(16 experts x 2 chunks x [32 gate/up + 16 down] N=512 bf16 matmuls =
1536 MMs) is the measured roofline; all DVE/ACT/DMA work hides under it
(a pure-MM skeleton of the same stream measures within ~7us, which is
the router's PE share).

Structure notes:
 - expert-0 chunk-0 gate/up is emitted BEFORE the router, so the PE
   starts as soon as x tile 0 + Wg[0] land (~1.5us) instead of after the
   full x DMA (5.6us); the router (which needs all of x) then runs while
   later loads are covered.
 - router PSUM borrows the psY rotation (those tiles are dead before the
   first down-tail needs them).
 - chunk pipeline: the tail (silu*up, down matmuls, weighted apply) of
   chunk c is emitted after gate/up of chunk c+1, giving the stage-2
   ACT/DVE ops a ~3.4us runway so single-buffered g/u PSUM never stalls
   the PE.
 - down PSUM is one bank per 512-wide half with the DVE apply issued per
   half, so the apply drain never blocks PE reuse of the bank.
 - y is produced in bf16: expert 15's apply writes a bf16 tile directly
   (DVE output cast) and each token tile is DMA'd as soon as its last
   apply finishes -- an ~11us exposed fp32 store tail becomes ~2us.
   The host upcasts to fp32 (adds <=2^-9 rounding, well inside the
   2e-2 budget; measured rel err 6.1e-3 vs 5.6e-3 for fp32 out).
"""

import numpy as np
import ml_dtypes

import concourse.bacc as bacc
import concourse.tile as tile
import concourse.mybir as mybir
from concourse import masks
from concourse.bass_utils import run_bass_kernel_spmd

B, T, D, E, K = 4, 2048, 1024, 16, 256
NCORES = 8
TOK = B * T // NCORES          # 1024 tokens per core
NT = TOK // 128                # 8 token tiles of 128
ND = D // 128                  # 8 d tiles
NK = K // 128                  # 2 k tiles
TC = 512                       # token chunk for expert stages
NC_CHUNK = TOK // TC           # 2 chunks
DOWN_FR = 512                  # down matmul moving width (1 PSUM bank max)
dt = mybir.dt
AF = mybir.ActivationFunctionType
ALU = mybir.AluOpType

_built = None


def _patch_hw_spec():
    pass


def _emit_body(nc, pools, dram, rep):
    res, rsb, wpool, s2p, psA, psY = pools
    xt, wr, ident, y_sb = dram["xt_t"], dram["wr_t"], dram["ident"], dram["y_sb"]
    wg_d, wu_d, wd_d = dram["wg_d"], dram["wu_d"], dram["wd_d"]
    y_d = dram["y_d"]

    # ---------- expert-stage emitters ----------
    def emit_gateup(wg_t, wu_t, c):
        g_ps = psA.tile([128, 2 * TC], dt.float32, tag="gu", name="g_ps")
        u_ps = psA.tile([128, 2 * TC], dt.float32, tag="gu", name="u_ps")
        for w_t, o_ps in ((wg_t, g_ps), (wu_t, u_ps)):
            for kt in range(NK):
                for a in range(ND):
                    nc.tensor.matmul(
                        o_ps[:, kt * TC:(kt + 1) * TC],
                        w_t[:, a * K + kt * 128: a * K + (kt + 1) * 128],
                        xt[:, a * TOK + c * TC: a * TOK + (c + 1) * TC],
                        start=(a == 0), stop=(a == ND - 1),
                    )
        return g_ps, u_ps

    def emit_stage2(g_ps, u_ps):
        # stage 2: hw = silu(g) * u   (hw cast to bf16 for the down mm)
        sg_t = s2p.tile([128, 2 * TC], dt.float32, tag="sgt", name="sg_t")
        nc.scalar.activation(sg_t[:], g_ps[:], AF.Silu)
        hw_t = s2p.tile([128, 2 * TC], dt.bfloat16, tag="hwt", name="hw_t")
        nc.vector.tensor_mul(hw_t[:], u_ps[:], sg_t[:])
        return hw_t

    def emit_down(e, c, hw_t, wd_t, w_all):

        # down: y[t,d] (+)= w[t,e] * (hw[k,t].T @ WdT[k,d])
        # one PSUM bank per dk-half, applied as soon as its 2 matmuls stop,
        # so the DVE drain never blocks the PE on psY reuse
        for tt in range(TC // 128):
            gt = c * (TC // 128) + tt      # global token tile
            ys = y_sb[gt]
            w_col = w_all[:, gt * E + e: gt * E + e + 1]
            y_bf = None
            if e == E - 1:
                y_bf = s2p.tile([128, D], dt.bfloat16, tag="ybf", name="y_bf")
            for dk in range(D // DOWN_FR):
                y_ps = psY.tile([128, DOWN_FR], dt.float32, tag="y",
                                name="y_ps")
                for kt in range(NK):
                    nc.tensor.matmul(
                        y_ps[:],
                        hw_t[:, kt * TC + tt * 128: kt * TC + (tt + 1) * 128],
                        wd_t[:, kt * D + dk * DOWN_FR:
                             kt * D + (dk + 1) * DOWN_FR],
                        start=(kt == 0), stop=(kt == NK - 1),
                    )
                sl = slice(dk * DOWN_FR, (dk + 1) * DOWN_FR)
                if e == 0:
                    nc.vector.tensor_scalar(ys[:, sl], y_ps[:], w_col, None,
                                            op0=ALU.mult)
                elif e == E - 1:
                    nc.vector.scalar_tensor_tensor(y_bf[:, sl], y_ps[:],
                                                   w_col, ys[:, sl],
                                                   op0=ALU.mult, op1=ALU.add)
                else:
                    nc.vector.scalar_tensor_tensor(ys[:, sl], y_ps[:], w_col,
                                                   ys[:, sl],
                                                   op0=ALU.mult, op1=ALU.add)
            if e == E - 1:
                nc.sync.dma_start(y_d[gt * 128:(gt + 1) * 128, :], y_bf[:])

    def emit_tail(e, c, g_ps, u_ps, wd_t, w_all):
        emit_down(e, c, emit_stage2(g_ps, u_ps), wd_t, w_all)

    # ---------- expert 0 chunk 0 gate/up first (needs only x tiles + Wg/Wu
    # of expert 0, so PE starts ~1.5us in while the rest of x streams) ----
    wts = {}
    for e in (0, 1):
        wg_t = wpool.tile([128, ND * K], dt.bfloat16, tag="wg", name="wg_t")
        nc.sync.dma_start(wg_t[:], wg_d[e])
        wu_t = wpool.tile([128, ND * K], dt.bfloat16, tag="wu", name="wu_t")
        nc.sync.dma_start(wu_t[:], wu_d[e])
        wd_t = wpool.tile([128, NK * D], dt.bfloat16, tag="wd", name="wd_t")
        nc.sync.dma_start(wd_t[:], wd_d[e])
        wts[e] = (wg_t, wu_t, wd_t)

    g00, u00 = emit_gateup(wts[0][0], wts[0][1], 0)
    hw00 = emit_stage2(g00, u00)   # silu/mul run during the router MMs,
                                   # freeing the psA rotation for chunk 1

    # ---------- router (batched), PSUM borrowed from psY's rotation ----
    # two col-group-tiled partial sums (4 accumulation steps each) run
    # CONCURRENTLY in different 32-col groups of the PE array, halving the
    # streamed column count; one DVE add combines them.
    lgT = rsb.tile([16, TOK], dt.float32, tag="lgT")
    for h in range(2):  # two 512-token halves
        lg_a = psY.tile([128, DOWN_FR], dt.float32, tag="y", name="lg_a")
        lg_b = psY.tile([128, DOWN_FR], dt.float32, tag="y", name="lg_b")
        for step in range(ND // 2):
            for g, (ps, base) in enumerate(((lg_a, 0), (lg_b, 32))):
                a = step + g * (ND // 2)
                nc.tensor.matmul(
                    ps[base:base + 16, :512],
                    wr[:, a * E:(a + 1) * E],
                    xt[:, a * TOK + h * 512: a * TOK + (h + 1) * 512],
                    start=(step == 0), stop=(step == ND // 2 - 1),
                    tile_position=(0, base),
                )
        lgB = rsb.tile([16, 512], dt.float32, tag="lgB", name="lgB")
        nc.scalar.copy(lgB[:], lg_b[32:48, :])
        nc.vector.tensor_add(lgT[:, h * 512:(h + 1) * 512],
                             lg_a[0:16, :], lgB[:])

    # chunk-1 gate/up streams while the DVE partial-adds finish
    g01, u01 = emit_gateup(wts[0][0], wts[0][1], 1)

    # transpose all 8 [16,128] slices into one [128, 8*16] tile
    ltr_ps = psY.tile([128, DOWN_FR], dt.float32, tag="y", name="ltr_ps")
    for tt in range(NT):
        nc.tensor.transpose(
            ltr_ps[:, tt * E:(tt + 1) * E],
            lgT[:, tt * 128:(tt + 1) * 128], ident[:16, :16])
    lg = rsb.tile([128, NT * E], dt.float32, tag="lg")
    nc.vector.tensor_copy(lg[:], ltr_ps[:, :NT * E])

    def v3(ap):  # [128, 8*16] -> [128, 8, 16]
        return ap.rearrange("p (a e) -> p a e", e=E)

    def bc(ap):  # [128, 8] -> [128, 8, 16] broadcast
        return ap.unsqueeze(2).broadcast_to([128, NT, E])

    m1 = rsb.tile([128, NT], dt.float32, tag="m1")
    nc.vector.reduce_max(m1[:], v3(lg[:]), axis=mybir.AxisListType.X)
    dd = rsb.tile([128, NT * E], dt.float32, tag="dd")
    nc.vector.tensor_sub(v3(dd[:]), v3(lg[:]), bc(m1[:]))
    eq = rsb.tile([128, NT * E], dt.float32, tag="eq")
    nc.vector.tensor_scalar(eq[:], dd[:], 0.0, None, op0=ALU.is_ge)
    msk = rsb.tile([128, NT * E], dt.float32, tag="msk")
    nc.vector.scalar_tensor_tensor(msk[:], eq[:], -1e30, lg[:],
                                   op0=ALU.mult, op1=ALU.add)
    thr = rsb.tile([128, NT], dt.float32, tag="thr")
    nc.vector.reduce_max(thr[:], v3(msk[:]), axis=mybir.AxisListType.X)
    sarg = rsb.tile([128, NT * E], dt.float32, tag="sarg")
    nc.vector.tensor_sub(v3(sarg[:]), v3(lg[:]), bc(thr[:]))
    sg = rsb.tile([128, NT * E], dt.float32, tag="sg")
    nc.scalar.activation(sg[:], sarg[:], AF.Sigmoid, scale=10.0)
    l2 = rsb.tile([128, NT * E], dt.float32, tag="l2")
    nc.vector.tensor_mul(l2[:], lg[:], sg[:])
    mx = rsb.tile([128, NT], dt.float32, tag="mx")
    nc.vector.reduce_max(mx[:], v3(l2[:]), axis=mybir.AxisListType.X)
    earg = rsb.tile([128, NT * E], dt.float32, tag="earg")
    nc.vector.tensor_sub(v3(earg[:]), v3(l2[:]), bc(mx[:]))
    ex = rsb.tile([128, NT * E], dt.float32, tag="ex")
    nc.scalar.activation(ex[:], earg[:], AF.Exp)
    sm = rsb.tile([128, NT], dt.float32, tag="sm")
    nc.vector.reduce_sum(sm[:], v3(ex[:]), axis=mybir.AxisListType.X)
    rs = rsb.tile([128, NT], dt.float32, tag="rs")
    nc.vector.reciprocal(rs[:], sm[:])
    # w_all[t, tt*16+e] = softmax weight; resident for the expert loop
    w_all = rsb.tile([128, NT * E], dt.float32, tag="w_all")
    nc.vector.tensor_mul(v3(w_all[:]), v3(ex[:]), bc(rs[:]))

    # ---------- expert loop (software-pipelined) ----------
    emit_down(0, 0, hw00, wts[0][2], w_all)
    pending = (0, 1, g01, u01, wts[0][2])
    for e in range(E):
        if e >= 2:
            wg_t = wpool.tile([128, ND * K], dt.bfloat16, tag="wg",
                              name="wg_t")
            nc.sync.dma_start(wg_t[:], wg_d[e])
            wu_t = wpool.tile([128, ND * K], dt.bfloat16, tag="wu",
                              name="wu_t")
            nc.sync.dma_start(wu_t[:], wu_d[e])
            wd_t = wpool.tile([128, NK * D], dt.bfloat16, tag="wd",
                              name="wd_t")
            nc.sync.dma_start(wd_t[:], wd_d[e])
        else:
            wg_t, wu_t, wd_t = wts[e]

        for c in range(NC_CHUNK):
            if e == 0:
                continue  # both chunks emitted above
            g_ps, u_ps = emit_gateup(wg_t, wu_t, c)
            if pending is not None:
                emit_tail(*pending, w_all)
            pending = (e, c, g_ps, u_ps, wd_t)
    emit_tail(*pending, w_all)


def _build(repeat=1, loop=1):
    _patch_hw_spec()
    nc = bacc.Bacc("TRN2", target_bir_lowering=False, debug=False,
                   num_devices=NCORES)

    xt_d = nc.dram_tensor("xt", (128, ND * TOK), dt.bfloat16,
                          kind="ExternalInput").ap()
    wg_d = nc.dram_tensor("wg", (E, 128, ND * K), dt.bfloat16,
                          kind="ExternalInput").ap()
    wu_d = nc.dram_tensor("wu", (E, 128, ND * K), dt.bfloat16,
                          kind="ExternalInput").ap()
    wd_d = nc.dram_tensor("wd", (E, 128, NK * D), dt.bfloat16,
                          kind="ExternalInput").ap()
    wr_d = nc.dram_tensor("wr", (128, ND * E), dt.bfloat16,
                          kind="ExternalInput").ap()
    y_d = nc.dram_tensor("y", (TOK, D), dt.bfloat16,
                         kind="ExternalOutput").ap()

    with tile.TileContext(nc) as tc:
        with (
            tc.tile_pool(name="resident", bufs=1) as res,
            tc.tile_pool(name="router_sb", bufs=2) as rsb,
            tc.tile_pool(name="wpool", bufs=6) as wpool,
            tc.tile_pool(name="stage2", bufs=3) as s2p,
            tc.tile_pool(name="psA", bufs=2, space="PSUM") as psA,
            tc.tile_pool(name="psY", bufs=4, space="PSUM") as psY,
        ):
            # ---------- resident loads ----------
            wr = res.tile([128, ND * E], dt.bfloat16, tag="wr")
            nc.sync.dma_start(wr[:], wr_d)
            xt = res.tile([128, ND * TOK], dt.bfloat16, tag="xt")
            for a in range(ND):
                nc.sync.dma_start(xt[:, a * TOK:(a + 1) * TOK],
                                  xt_d[:, a * TOK:(a + 1) * TOK])
            ident = res.tile([128, 128], dt.float32, tag="ident")
            masks.make_identity(nc, ident[:])
            y_sb = [res.tile([128, D], dt.float32, tag=f"ysb{i}",
                             name=f"ysb{i}") for i in range(NT)]

            pools = (res, rsb, wpool, s2p, psA, psY)
            dram = dict(xt_t=xt, wr_t=wr, ident=ident, y_sb=y_sb,
                        wg_d=wg_d, wu_d=wu_d, wd_d=wd_d, y_d=y_d)
            if loop > 1:
                with tc.For_i(0, loop, 1):
                    for _ in range(repeat):
                        _emit_body(nc, pools, dram, _)
            else:
                for _ in range(repeat):
                    _emit_body(nc, pools, dram, _)

    nc.compile()
    return nc


def _prep_inputs(x, W_up, W_gate, W_down, W_router, log_temp):
    """Host-side repack: fold temp, transpose weights, partition-major,
    cast to bf16."""
    bf = ml_dtypes.bfloat16
    x = np.asarray(x, dtype=np.float32)
    W_up = np.asarray(W_up, dtype=np.float32)
    W_gate = np.asarray(W_gate, dtype=np.float32)
    W_down = np.asarray(W_down, dtype=np.float32)
    W_router = np.asarray(W_router, dtype=np.float32)
    lt = float(np.asarray(log_temp, dtype=np.float32))
    temp = float(np.log1p(np.exp(lt)) + 0.1)

    X = x.reshape(B * T, D)
    xT = np.ascontiguousarray(X.T)                       # [D, 8192]
    xt_cores = []
    for c in range(NCORES):
        sl = xT[:, c * TOK:(c + 1) * TOK]                # [1024, 1024]
        xt_cores.append(np.ascontiguousarray(
            sl.reshape(ND, 128, TOK).transpose(1, 0, 2)
            .reshape(128, ND * TOK)).astype(bf))

    def pack_dk(w):  # [E, K, D] (torch [out,in]) -> [E, 128, 8*K], d-major
        wt = w.transpose(0, 2, 1)                        # [E, D, K]
        return np.ascontiguousarray(
            wt.reshape(E, ND, 128, K).transpose(0, 2, 1, 3)
            .reshape(E, 128, ND * K)).astype(bf)

    wg_h = pack_dk(W_gate)
    wu_h = pack_dk(W_up)
    wdt = W_down.transpose(0, 2, 1)                      # [E, K, D]
    wd_h = np.ascontiguousarray(
        wdt.reshape(E, NK, 128, D).transpose(0, 2, 1, 3)
        .reshape(E, 128, NK * D)).astype(bf)
    wrt = np.ascontiguousarray(W_router.T) / temp        # [D, E]
    wr_h = np.ascontiguousarray(
        wrt.reshape(ND, 128, E).transpose(1, 0, 2)
        .reshape(128, ND * E)).astype(bf)

    in_maps = []
    for c in range(NCORES):
        in_maps.append({
            "xt": xt_cores[c],
            "wg": wg_h, "wu": wu_h, "wd": wd_h, "wr": wr_h,
        })
    return in_maps


def kernel(x, W_up, W_gate, W_down, W_router, log_temp, _trace=False):
    global _built
    if _built is None:
        _built = _build()
    nc = _built
    in_maps = _prep_inputs(x, W_up, W_gate, W_down, W_router, log_temp)
    res = run_bass_kernel_spmd(nc, in_maps, core_ids=list(range(NCORES)),
                               trace=_trace)
    out = np.empty((B * T, D), dtype=np.float32)
    for c in range(NCORES):
        out[c * TOK:(c + 1) * TOK, :] = res.results[c]["y"].astype(np.float32)
    kernel.last_results = res
    return out.reshape(B, T, D)
